# revision 1
# baseline (speedup 1.0000x reference)
"""DynamicCenterLoss on Trainium2 (Bass/Tile), 8-core SPMD.

Strategy: `batch` is sorted, so shard at batch boundaries -> core b owns
batch b (sizes ~N/8 +- <1%). Per core, every needed statistic is a
13-class one-hot segment reduction computed on the tensor engine:

    OUT[13, 65] = sum_n onehot(tgt_n)^T (x) [feat_n | 1]
      -> fsum[13,64] (per-class feature sums), ccnt[13] (per-class counts)

plus S = sum_n ||feat_n||^2 via ScalarE Square+accumulate.  The intra
term uses  sum_n ||f_n - c_{t_n}||^2 = S - 2*sum_c c_c.fsum_c + sum_c
ccnt_c*||c_c||^2, so no per-point gather of centers is ever needed.
Padded rows (target=13) produce an all-zero one-hot row and zero
features, so they contribute nothing. The pairwise-center hinge loss is
computed per core on its own (13,64) stats; the host only averages the
8 per-batch scalars.
"""

import numpy as np

import concourse.bass as bass
import concourse.bacc as bacc
import concourse.tile as tile
from concourse import mybir
from concourse.bass_utils import run_bass_kernel_spmd

P = 128
D = 64
C = 13
B = 8
N_CORES = 8
MARGIN = 0.5
INTRA_W = 1.0
INTER_W = 1.0
LOSS_W = 0.01
IGNORE = -1
TT = 64  # matmul steps (128-point chunks) per SBUF tile

f32 = mybir.dt.float32
bf16 = mybir.dt.bfloat16
i32 = mybir.dt.int32


def build_nc(T: int) -> bass.Bass:
    """Build the per-core Bass program. T = points per SBUF partition."""
    Npad = P * T
    # tile splits: small first tiles so the PE pipeline fills early
    splits = []
    _t0 = 0
    szs = [16, 48]
    while sum(szs) + TT <= T:
        szs.append(TT)
    for sz in szs:
        if _t0 >= T:
            break
        sz = min(sz, T - _t0)
        splits.append((_t0, sz))
        _t0 += sz
    if _t0 < T:
        splits.append((_t0, T - _t0))
    ntiles = len(splits)

    nc = bacc.Bacc("TRN2", target_bir_lowering=False)
    feat_h = nc.dram_tensor("feat", [Npad, D], f32, kind="ExternalInput")
    tgt_h = nc.dram_tensor("tgt", [Npad], i32, kind="ExternalInput")
    cen_h = nc.dram_tensor("centers", [C, D], f32, kind="ExternalInput")
    out_h = nc.dram_tensor("out", [1, 8], f32, kind="ExternalOutput")

    # point n == (p, t) with n = p*T + t  -> per-partition contiguous DMA
    featv = feat_h[:, :].rearrange("(p t) d -> p t d", p=P)  # [128, T, 64]
    tgtv = tgt_h[:].rearrange("(p t) -> p t", p=P)  # [128, T]

    with tile.TileContext(nc) as tc:
        with (
            tc.tile_pool(name="consts", bufs=1) as cp,
            tc.tile_pool(name="io", bufs=6) as iop,
            tc.tile_pool(name="ex", bufs=4) as exp_,
            tc.tile_pool(name="oh", bufs=3) as ohp,
            tc.tile_pool(name="sq", bufs=2) as sqp,
            tc.tile_pool(name="acc", bufs=1, space="PSUM") as psa,
            tc.tile_pool(name="ps2", bufs=1, space="PSUM") as ps2,
            tc.tile_pool(name="fin", bufs=1) as fp,
        ):
            # ---- constants ----
            iota_rep = cp.tile([P, TT, C], i32)
            nc.gpsimd.iota(
                iota_rep[:, :, :], pattern=[[0, TT], [1, C]], base=0,
                channel_multiplier=0,
            )
            tgt_sb = cp.tile([P, T], i32)
            nc.scalar.dma_start(out=tgt_sb[:, :], in_=tgtv[:, :])
            cen_sb = cp.tile([C, D], f32)
            nc.scalar.dma_start(out=cen_sb[:, :], in_=cen_h[:, :])
            ones = cp.tile([P, 1], f32)
            nc.vector.memset(ones[:, :], 1.0)
            warm = cp.tile([1, 1], f32)
            nc.scalar.activation(
                out=warm[:, :], in_=ones[0:1, :],
                func=mybir.ActivationFunctionType.Sqrt,
            )
            ident = cp.tile([C, C], f32)
            nc.vector.memset(ident[:, :], 1.0)
            nc.gpsimd.affine_select(
                out=ident[:, :], in_=ident[:, :],
                compare_op=mybir.AluOpType.is_equal, fill=0.0,
                base=0, pattern=[[-1, C]], channel_multiplier=1,
            )
            bigeye = cp.tile([1, C, C], f32)
            nc.vector.memset(bigeye[:, :, :], 1e6)
            nc.gpsimd.affine_select(
                out=bigeye[:, :, :], in_=bigeye[:, :, :],
                compare_op=mybir.AluOpType.is_equal, fill=0.0,
                base=0, pattern=[[1, C], [-1, C]], channel_multiplier=0,
            )
            sq_acc = cp.tile([P, ntiles], f32)
            # identity rows at partitions [GRP, GRP+C): lhsT for merging
            # the second column-group accumulator
            GRP = 32
            eye_b = cp.tile([GRP + C, C], f32)
            nc.vector.memset(eye_b[:, :], 1.0)
            nc.gpsimd.affine_select(
                out=eye_b[:, :], in_=eye_b[:, :],
                compare_op=mybir.AluOpType.is_equal, fill=0.0,
                base=-GRP, pattern=[[-1, C]], channel_multiplier=1,
            )

            # ---- main loop: accumulate OUT[13, 65] over all points ----
            # two accumulators in different PE column groups so each
            # chunk's LDWEIGHTS overlaps the other group's MATMUL
            acc0 = psa.tile([C, D + 1], f32)
            acc1 = psa.tile([GRP + C, D + 1], f32)
            accs = [acc0[:, :], acc1[GRP : GRP + C, :]]
            last_step = [-1, -1]
            s = 0
            for _, tt in splits:
                for t in range(tt):
                    last_step[s % 2] = s
                    s += 1
            step = 0
            started = [False, False]
            for i, (t0, tt) in enumerate(splits):
                # dense f32 load (16KB+ contiguous per partition), then
                # DVE-cast to bf16 into the [feat | 1] layout for the PE
                f32t = iop.tile([P, TT, D], f32, tag="f32t")
                nc.sync.dma_start(
                    out=f32t[:, :tt, :], in_=featv[:, t0 : t0 + tt, :]
                )
                ext = exp_.tile([P, TT, D + 1], bf16, tag="ext")
                nc.vector.memset(ext[:, :tt, D : D + 1], 1.0)
                nc.vector.tensor_copy(ext[:, :tt, 0:D], f32t[:, :tt, :])
                oh = ohp.tile([P, TT, C], bf16, tag="oh")
                nc.vector.tensor_tensor(
                    out=oh[:, :tt, :],
                    in0=tgt_sb[:, t0 : t0 + tt].unsqueeze(2).to_broadcast(
                        [P, tt, C]
                    ),
                    in1=iota_rep[:, :tt, :],
                    op=mybir.AluOpType.is_equal,
                )
                sq = sqp.tile([P, TT, D], bf16, tag="sq")
                nc.scalar.activation(
                    out=sq[:, :tt, :], in_=f32t[:, :tt, :],
                    func=mybir.ActivationFunctionType.Square,
                    accum_out=sq_acc[:, i : i + 1],
                )
                for t in range(tt):
                    g = step % 2
                    nc.tensor.matmul(
                        accs[g],
                        lhsT=oh[:, t, :],
                        rhs=ext[:, t, :],
                        start=not started[g],
                        stop=(step == last_step[g]),
                        tile_position=(0, g * GRP),
                    )
                    started[g] = True
                    step += 1

            # ---- finale (tiny, per-core) ----
            c0 = fp.tile([C, D + 1], f32)
            nc.vector.tensor_copy(c0[:, :], acc0[:, :])
            c1 = fp.tile([GRP + C, D + 1], f32)
            nc.vector.tensor_copy(
                c1[GRP : GRP + C, :], acc1[GRP : GRP + C, :]
            )
            acc = ps2.tile([C, D + 1], f32)
            nc.tensor.matmul(
                acc[:, :], lhsT=ident[:, :], rhs=c0[:, :],
                start=True, stop=False,
            )
            nc.tensor.matmul(
                acc[:, :], lhsT=eye_b[GRP : GRP + C, :],
                rhs=c1[GRP : GRP + C, :], start=False, stop=True,
            )
            fsum = acc[:, 0:D]  # [13, 64]
            ccnt = acc[:, D : D + 1]  # [13, 1]

            # per-class means and presence
            cmax = fp.tile([C, 1], f32)
            nc.vector.tensor_scalar(
                out=cmax[:, :], in0=ccnt, scalar1=1.0, scalar2=None,
                op0=mybir.AluOpType.max,
            )
            rec = fp.tile([C, 1], f32)
            nc.vector.reciprocal(rec[:, :], cmax[:, :])
            trin = fp.tile([C, D], f32)
            nc.vector.tensor_scalar(
                out=trin[:, :], in0=fsum, scalar1=rec[:, :], scalar2=None,
                op0=mybir.AluOpType.mult,
            )
            pres = fp.tile([C, 1], f32)
            nc.vector.tensor_scalar(
                out=pres[:, :], in0=ccnt, scalar1=0.0,
                scalar2=None, op0=mybir.AluOpType.is_gt,
            )

            # per-class dot(centers, fsum), ccnt*||c||^2  -> pack3
            junk0 = fp.tile([C, D], f32)
            cn2 = fp.tile([C, 1], f32)
            nc.vector.tensor_tensor(
                out=junk0[:, :], in0=cen_sb[:, :], in1=cen_sb[:, :],
                op=mybir.AluOpType.mult,
            )
            nc.vector.tensor_reduce(
                out=cn2[:, :], in_=junk0[:, :],
                axis=mybir.AxisListType.X, op=mybir.AluOpType.add,
            )
            pack3 = fp.tile([C, 3], f32)
            junk1 = fp.tile([C, D], f32)
            nc.vector.tensor_tensor(
                out=junk1[:, :], in0=cen_sb[:, :], in1=fsum,
                op=mybir.AluOpType.mult,
            )
            nc.vector.tensor_reduce(
                out=pack3[:, 0:1], in_=junk1[:, :],
                axis=mybir.AxisListType.X, op=mybir.AluOpType.add,
            )
            nc.vector.tensor_tensor(
                out=pack3[:, 1:2], in0=cn2[:, :], in1=ccnt,
                op=mybir.AluOpType.mult,
            )
            nc.vector.tensor_copy(pack3[:, 2:3], ccnt)

            # cross-partition sums over the 13 classes: [Tdot, Utot, cnt_b]
            red3 = ps2.tile([1, 3], f32)
            nc.tensor.matmul(
                red3[:, :], lhsT=ones[0:C, :], rhs=pack3[:, :],
                start=True, stop=True,
            )

            # S = sum over all partitions/tiles of sq_acc
            red_sq = fp.tile([P, 1], f32)
            nc.vector.tensor_reduce(
                out=red_sq[:, :], in_=sq_acc[:, :],
                axis=mybir.AxisListType.X, op=mybir.AluOpType.add,
            )
            s_ps = ps2.tile([1, 1], f32)
            nc.tensor.matmul(
                s_ps[:, :], lhsT=ones[:, :], rhs=red_sq[:, :],
                start=True, stop=True,
            )

            # transpose cmeans -> [64, 13]; present -> [1, 13]
            trps = ps2.tile([D, C], f32)
            nc.tensor.transpose(trps[:, :], trin[:, :], ident[:, :])
            trsb = fp.tile([D, C], f32)
            nc.vector.tensor_copy(trsb[:, :], trps[:, :])
            cmT = trsb[0:D, :]  # [64, 13]
            prps = ps2.tile([1, C], f32)
            nc.tensor.transpose(prps[:, :], pres[:, :], ident[:, :])
            presT = fp.tile([1, C], f32)
            nc.vector.tensor_copy(presT[:, :], prps[:, :])

            # pairwise squared distances between class means
            diff = fp.tile([D, C, C], f32)
            nc.vector.tensor_tensor(
                out=diff[:, :, :],
                in0=cmT.unsqueeze(2).to_broadcast([D, C, C]),
                in1=cmT.unsqueeze(1).to_broadcast([D, C, C]),
                op=mybir.AluOpType.subtract,
            )
            dsq = fp.tile([D, C, C], f32)
            nc.vector.tensor_tensor(
                out=dsq[:, :, :], in0=diff[:, :, :], in1=diff[:, :, :],
                op=mybir.AluOpType.mult,
            )
            dd2 = ps2.tile([1, C * C], f32)
            nc.tensor.matmul(
                dd2[:, :], lhsT=ones[0:1, :],
                rhs=bigeye[:, :, :].rearrange("p a b -> p (a b)"),
                start=True, stop=False,
            )
            nc.tensor.matmul(
                dd2[:, :], lhsT=ones[0:D, :],
                rhs=dsq[:, :, :].rearrange("d a b -> d (a b)"),
                start=False, stop=True,
            )
            dist = fp.tile([1, C * C], f32)
            nc.scalar.activation(
                out=dist[:, :], in_=dd2[:, :],
                func=mybir.ActivationFunctionType.Sqrt,
            )
            hinge = fp.tile([1, C * C], f32)  # holds -relu(M - dist)
            nc.vector.tensor_scalar(
                out=hinge[:, :], in0=dist[:, :], scalar1=MARGIN,
                scalar2=MARGIN, op0=mybir.AluOpType.min,
                op1=mybir.AluOpType.subtract,
            )
            pm = fp.tile([1, C, C], f32)
            nc.vector.tensor_tensor(
                out=pm[:, :, :],
                in0=presT[:, :].unsqueeze(2).to_broadcast([1, C, C]),
                in1=presT[:, :].unsqueeze(1).to_broadcast([1, C, C]),
                op=mybir.AluOpType.mult,
            )
            pmf = pm[:, :, :].rearrange("p a b -> p (a b)")
            # raw per-batch sums; host does the final few divisions
            scal = fp.tile([1, 8], f32)
            nc.vector.memset(scal[:, 6:8], 0.0)
            terms = fp.tile([1, C * C], f32)
            nc.vector.tensor_tensor(
                out=terms[:, :], in0=hinge[:, :], in1=pmf,
                op=mybir.AluOpType.mult,
            )
            nc.vector.tensor_reduce(
                out=scal[:, 4:5], in_=terms[:, :],
                axis=mybir.AxisListType.X, op=mybir.AluOpType.add,
            )
            nc.vector.tensor_reduce(
                out=scal[:, 5:6], in_=presT[:, :], axis=mybir.AxisListType.X,
                op=mybir.AluOpType.add,
            )
            nc.vector.tensor_copy(scal[:, 0:1], s_ps[:, :])
            nc.vector.tensor_copy(scal[:, 1:4], red3[:, :])

            nc.sync.dma_start(out=out_h[:, :], in_=scal[:, :])
    nc.finalize()
    return nc


# set by test.py to capture profile info
TRACE = False
LAST = {}


def _ensure_ntff_hook():
    """The agent image's antenv lacks axon_hooks; synthesize it so
    run_bass_kernel_spmd(trace=True) can profile. Best-effort."""
    import sys
    import types

    try:
        from antenv.axon_hooks import get_axon_ntff_profile_hook  # noqa: F401
        return
    except ImportError:
        pass
    try:
        from trn_agent_boot.trn_boot import _ntff_profile_via_ctypes

        hook = _ntff_profile_via_ctypes("/opt/axon/libaxon_pjrt.so")
        mod = types.ModuleType("antenv.axon_hooks")
        mod._hook = hook
        mod.get_axon_ntff_profile_hook = lambda: mod._hook
        mod.set_axon_ntff_profile_hook = lambda h: setattr(mod, "_hook", h)
        sys.modules["antenv.axon_hooks"] = mod
        import antenv

        antenv.axon_hooks = mod
    except Exception as e:  # degrade: no profile, run still works
        print(f"ntff hook injection failed: {e}")


def kernel(pred=None, target=None, feat=None, batch=None, centers=None):
    target = np.asarray(target)
    feat = np.asarray(feat, dtype=np.float32)
    batch = np.asarray(batch)
    centers = np.asarray(centers, dtype=np.float32)
    N = feat.shape[0]

    # shard at batch boundaries: core b <- batch b (batch is sorted)
    bounds = np.searchsorted(batch, np.arange(B + 1))
    sizes = np.diff(bounds)
    T = int(max((int(sizes.max()) + P - 1) // P, TT))
    Npad = P * T

    in_maps = []
    for b in range(B):
        lo, hi = int(bounds[b]), int(bounds[b + 1])
        fb = np.zeros((Npad, D), dtype=np.float32)
        tb = np.full((Npad,), C, dtype=np.int32)
        fb[: hi - lo] = feat[lo:hi]
        tb[: hi - lo] = target[lo:hi]
        inv = tb == IGNORE
        if inv.any():
            tb[inv] = C  # one-hot miss -> excluded everywhere
            fb[inv] = 0.0  # excluded from S
        in_maps.append({"feat": fb, "tgt": tb, "centers": centers})

    nc = build_nc(T)
    if TRACE:
        _ensure_ntff_hook()
    res = run_bass_kernel_spmd(nc, in_maps, list(range(N_CORES)), trace=TRACE)
    LAST["results"] = res

    rows = np.stack(
        [np.asarray(res.results[b]["out"]).reshape(8) for b in range(B)]
    ).astype(np.float64)
    s, tdot, utot, cnt_b, tsum, kpres = (rows[:, j] for j in range(6))
    npairs = kpres * (kpres - 1.0)
    intra = (s - 2.0 * tdot + utot) / np.maximum(cnt_b, 1.0)
    inter = -tsum / np.maximum(npairs, 1.0)
    present = cnt_b > 0
    den = max(float(present.sum()), 1.0)
    loss = LOSS_W * (
        INTRA_W * float(np.where(present, intra, 0.0).sum()) / den
        + INTER_W * float(np.where(present, inter, 0.0).sum()) / den
    )
    return np.float32(loss)



# revision 5
# speedup vs baseline: 1.6583x; 1.6583x over previous
"""DynamicCenterLoss on Trainium2 (Bass/Tile), 8-core SPMD.

Strategy: `batch` is sorted, so shard at batch boundaries -> core b owns
batch b. Host uploads ONE fp8 tensor per core, [Npad, 77] =
[feat_fp8(64) | onehot_fp8(13)] (4x less HBM than f32 feat, and no
on-device cast / broadcast-is_equal).  Device computes only the O(N)
reductions:

  - fsum[13,64] per-class feature sums: per-128-point chunk matmul
    acc += onehot^T @ feat, spread over 4 PE column groups
    (tile_position) so 4 chunk-matmuls stream concurrently.
  - S = sum ||f||^2: fused square+reduce split across ScalarE
    (activation Square + accum_out, cols 0:36) and the DVE
    (tensor_tensor_reduce in0*in1 + accum, cols 36:64), so the two
    engines each carry ~half of the 4.2M-element square pass.

Counts/presence (ccnt, cnt_b) are exact host-side bincounts of
target/batch; the tiny (13,64) pairwise-center tail runs on the host in
f64 as part of the gather.  fp8e4 quantization of feat gives rel err
~3.5e-4 on the loss (tolerance 2e-2).
"""

import numpy as np
import ml_dtypes

import concourse.bass as bass
import concourse.bacc as bacc
import concourse.tile as tile
from concourse import mybir
from concourse.bass_utils import run_bass_kernel_spmd

P = 128
D = 64
C = 13
W = D + C  # 77: feat | onehot
B = 8
N_CORES = 8
MARGIN = 0.5
INTRA_W = 1.0
INTER_W = 1.0
LOSS_W = 0.01
IGNORE = -1
SA = 35  # feat columns squared on ScalarE; the rest (D-SA) on the DVE
NGRP = 4  # PE column groups

f32 = mybir.dt.float32
f8 = mybir.dt.float8e4
i32 = mybir.dt.int32

F8NP = getattr(ml_dtypes, "float8_e4m3", ml_dtypes.float8_e4m3fn)


def _splits(T: int) -> list[tuple[int, int]]:
    """Tile sizes: small first tile (compute starts early) and small last
    tile (less square-work exposed after the DMA stream ends)."""
    szs = []
    rem = T
    for s in (32, 96):
        t = min(s, rem)
        if t > 0:
            szs.append(t)
            rem -= t
    while rem > 192:
        szs.append(128)
        rem -= 128
    if rem > 64:
        szs.append(rem - 32)
        rem = 32
    if rem > 0:
        szs.append(rem)
    out = []
    t0 = 0
    for sz in szs:
        out.append((t0, sz))
        t0 += sz
    return out


def build_nc(T: int) -> bass.Bass:
    splits = _splits(T)
    ntiles = len(splits)
    K = 2 * ntiles  # accumulator columns (ScalarE + DVE per tile)
    TTMAX = max(tt for _, tt in splits)
    Npad = P * T

    nc = bacc.Bacc("TRN2", target_bir_lowering=False)
    comb_h = nc.dram_tensor("comb", [Npad, W], f8, kind="ExternalInput")
    out_h = nc.dram_tensor("out", [P, D + K], f32, kind="ExternalOutput")

    # point n == (p, t) with n = p*T + t -> per-partition contiguous DMA
    combv = comb_h[:, :].rearrange("(p t) d -> p t d", p=P)  # [128, T, 77]

    # accumulation-group bookkeeping: chunk s -> column group s % NGRP
    first = [min(s for s in range(T) if s % NGRP == g) for g in range(NGRP)]
    last = [max(s for s in range(T) if s % NGRP == g) for g in range(NGRP)]

    with tile.TileContext(nc) as tc:
        with (
            tc.tile_pool(name="fin", bufs=1) as fp,
            tc.tile_pool(name="io", bufs=3) as iop,
            tc.tile_pool(name="sqa", bufs=2) as sap,
            tc.tile_pool(name="sqv", bufs=2) as svp,
            tc.tile_pool(name="acc", bufs=1, space="PSUM") as psa,
        ):
            final = fp.tile([P, D + K], f32)
            nc.vector.memset(final[:, :], 0.0)
            # warm the Square activation table during the first DMA
            warm = fp.tile([1, 1], f32)
            nc.scalar.activation(
                out=warm[:, :], in_=final[0:1, 0:1],
                func=mybir.ActivationFunctionType.Square,
            )

            acc = psa.tile([32 * (NGRP - 1) + C, D], f32)  # [109, 64]

            step = 0
            for i, (t0, tt) in enumerate(splits):
                io = iop.tile([P, TTMAX, W], f8, tag="io")
                nc.sync.dma_start(
                    out=io[:, :tt, :], in_=combv[:, t0 : t0 + tt, :]
                )
                sa = sap.tile([P, TTMAX, SA], f8, tag="sa")
                nc.scalar.activation(
                    out=sa[:, :tt, :], in_=io[:, :tt, 0:SA],
                    func=mybir.ActivationFunctionType.Square,
                    accum_out=final[:, D + 2 * i : D + 2 * i + 1],
                )
                sv = svp.tile([P, TTMAX, D - SA], f8, tag="sv")
                nc.vector.affine_mul_reduce(
                    out=sv[:, :tt, :],
                    accum_out=final[:, D + 2 * i + 1 : D + 2 * i + 2],
                    in0=io[:, :tt, SA:D],
                    in1=io[:, :tt, SA:D],
                    scale=1.0,
                    bias=0.0,
                )
                for t in range(tt):
                    g = step % NGRP
                    nc.tensor.matmul(
                        acc[32 * g : 32 * g + C, :],
                        lhsT=io[:, t, D:W],
                        rhs=io[:, t, 0:D],
                        start=(step == first[g]),
                        stop=(step == last[g]),
                        tile_position=(0, 32 * g),
                    )
                    step += 1

            for g in range(NGRP):
                nc.vector.tensor_copy(
                    final[32 * g : 32 * g + C, 0:D],
                    acc[32 * g : 32 * g + C, :],
                )
            nc.sync.dma_start(out=out_h[:, :], in_=final[:, :])
    nc.finalize()
    return nc


# set by test.py to capture profile info
TRACE = False
LAST = {}


def _ensure_ntff_hook():
    """The agent image's antenv lacks axon_hooks; synthesize it so
    run_bass_kernel_spmd(trace=True) can profile. Best-effort."""
    import sys
    import types

    try:
        from antenv.axon_hooks import get_axon_ntff_profile_hook  # noqa: F401
        return
    except ImportError:
        pass
    try:
        from trn_agent_boot.trn_boot import _ntff_profile_via_ctypes

        hook = _ntff_profile_via_ctypes("/opt/axon/libaxon_pjrt.so")
        mod = types.ModuleType("antenv.axon_hooks")
        mod._hook = hook
        mod.get_axon_ntff_profile_hook = lambda: mod._hook
        mod.set_axon_ntff_profile_hook = lambda h: setattr(mod, "_hook", h)
        sys.modules["antenv.axon_hooks"] = mod
        import antenv

        antenv.axon_hooks = mod
    except Exception as e:  # degrade: no profile, run still works
        print(f"ntff hook injection failed: {e}")


def kernel(pred=None, target=None, feat=None, batch=None, centers=None):
    target = np.asarray(target)
    feat = np.asarray(feat, dtype=np.float32)
    batch = np.asarray(batch)
    centers = np.asarray(centers, dtype=np.float64)
    N = feat.shape[0]

    # shard at batch boundaries: core b <- batch b (batch is sorted)
    bounds = np.searchsorted(batch, np.arange(B + 1))
    sizes = np.diff(bounds)
    T = int(max((int(sizes.max()) + P - 1) // P, 4))
    Npad = P * T

    valid_all = target != IGNORE
    feat8 = np.clip(feat, -240.0, 240.0).astype(F8NP)

    in_maps = []
    for b in range(B):
        lo, hi = int(bounds[b]), int(bounds[b + 1])
        n = hi - lo
        comb = np.zeros((Npad, W), dtype=F8NP)
        v = valid_all[lo:hi]
        fb = feat8[lo:hi].copy()
        fb[~v] = 0
        comb[:n, 0:D] = fb
        tb = target[lo:hi]
        rows = np.nonzero(v)[0]
        comb[rows, D + tb[rows]] = 1.0
        in_maps.append({"comb": comb})

    nc = build_nc(T)
    if TRACE:
        _ensure_ntff_hook()
    res = run_bass_kernel_spmd(nc, in_maps, list(range(N_CORES)), trace=TRACE)
    LAST["results"] = res

    # host-side (exact, from int inputs only): per-batch/class counts
    seg = (batch.astype(np.int64) * C + np.where(valid_all, target, 0))[
        valid_all
    ]
    ccnt = np.bincount(seg, minlength=B * C).reshape(B, C).astype(np.float64)
    cnt_b = np.bincount(batch[valid_all], minlength=B).astype(np.float64)

    cen_sq = (centers**2).sum(axis=1)  # (13,)
    total_intra = 0.0
    total_inter = 0.0
    n_present = 0
    for b in range(B):
        o = np.asarray(res.results[b]["out"]).astype(np.float64)  # [128,64+K]
        fsum = np.zeros((C, D))
        for g in range(NGRP):
            fsum += o[32 * g : 32 * g + C, 0:D]
        S = o[:, D:].sum()
        if cnt_b[b] <= 0:
            continue
        n_present += 1
        tdot = (centers * fsum).sum()
        utot = (ccnt[b] * cen_sq).sum()
        total_intra += (S - 2.0 * tdot + utot) / max(cnt_b[b], 1.0)

        cm = fsum / np.maximum(ccnt[b], 1.0)[:, None]
        pres = ccnt[b] > 0
        dd2 = ((cm[:, None, :] - cm[None, :, :]) ** 2).sum(-1)
        pm = pres[:, None] & pres[None, :] & ~np.eye(C, dtype=bool)
        dist = np.sqrt(np.where(pm, dd2, 1.0))
        terms = np.where(pm, np.maximum(MARGIN - dist, 0.0), 0.0)
        npairs = pm.sum()
        total_inter += terms.sum() / max(npairs, 1)

    den = max(n_present, 1)
    loss = LOSS_W * (
        INTRA_W * total_intra / den + INTER_W * total_inter / den
    )
    return np.float32(loss)


# revision 6
# speedup vs baseline: 1.7377x; 1.0478x over previous
"""DynamicCenterLoss on Trainium2 (Bass/Tile), 8-core SPMD.

Strategy: `batch` is sorted, so shard at batch boundaries -> core b owns
batch b. Host uploads two fp8 tensors per core: feat [Npad, 64] and
onehot(target) [Npad, 13] (4x less HBM than f32 feat; no on-device cast
or broadcast-is_equal).  Device computes only the O(N) reductions:

  - fsum[13,64] per-class feature sums: per-128-point chunk matmul
    acc += onehot^T @ feat, spread over 4 PE column groups
    (tile_position) so 4 chunk-matmuls stream concurrently.
  - S = sum ||f||^2: each DMA tile is squared+reduced by ONE engine --
    ScalarE (activation Square + accum_out) or the DVE (custom-DVE
    affine_mul_reduce in0*in1 + accum; the native tensor_tensor_reduce
    ISA op crashes this silicon) -- with tile ownership balanced by the
    engines' 1.2 vs 0.96 GHz rates.  Flat step-1 access patterns.

Counts/presence (ccnt, cnt_b) are exact host-side bincounts of
target/batch; the tiny (13,64) pairwise-center tail runs on the host in
f64 as part of the gather.  fp8e4 quantization of feat gives rel err
~3.5e-4 on the loss (tolerance 2e-2).
"""

import numpy as np
import ml_dtypes

import concourse.bass as bass
import concourse.bacc as bacc
import concourse.tile as tile
from concourse import mybir
from concourse.bass_utils import run_bass_kernel_spmd

P = 128
D = 64
C = 13
B = 8
N_CORES = 8
MARGIN = 0.5
INTRA_W = 1.0
INTER_W = 1.0
LOSS_W = 0.01
IGNORE = -1
NGRP = 4  # PE column groups

f32 = mybir.dt.float32
f8 = mybir.dt.float8e4

F8NP = getattr(ml_dtypes, "float8_e4m3", ml_dtypes.float8_e4m3fn)


def _plan(T: int) -> list[tuple[int, int, str]]:
    """(t0, tt, owner) tiles. Small first tile (compute starts early),
    small last tile (less square work exposed after the DMA stream);
    owners greedy-balanced for ScalarE 1.2 GHz vs DVE 0.96 GHz."""
    szs = []
    rem = T
    for s in (32, 96):
        t = min(s, rem)
        if t:
            szs.append(t)
            rem -= t
    while rem > 160:
        szs.append(128)
        rem -= 128
    if rem > 32:
        szs.append(rem - 24)
        rem = 24
    if rem:
        szs.append(rem)

    out = []
    t0 = 0
    ts = tv = 0.0  # projected engine-busy ns
    for sz in szs:
        cs = ts + sz * D / 1.2 + 640  # init + read-accumulator
        cv = tv + sz * D / 0.96 + 250
        if cs <= cv:
            out.append((t0, sz, "S"))
            ts = cs
        else:
            out.append((t0, sz, "V"))
            tv = cv
        t0 += sz
    return out


def build_nc(T: int) -> bass.Bass:
    plan = _plan(T)
    ntiles = len(plan)
    K = ntiles  # one accumulator column per tile
    TTMAX = max(tt for _, tt, _ in plan)
    Npad = P * T

    nc = bacc.Bacc("TRN2", target_bir_lowering=False)
    feat_h = nc.dram_tensor("feat8", [Npad, D], f8, kind="ExternalInput")
    oh_h = nc.dram_tensor("oh8", [Npad, C], f8, kind="ExternalInput")
    out_h = nc.dram_tensor("out", [P, D + K], f32, kind="ExternalOutput")

    # point n == (p, t), n = p*T + t -> per-partition contiguous, flat
    featv = feat_h[:, :].rearrange("(p t) d -> p (t d)", p=P)  # [128, T*64]
    ohv = oh_h[:, :].rearrange("(p t) d -> p (t d)", p=P)  # [128, T*13]

    first = [min(s for s in range(T) if s % NGRP == g) for g in range(NGRP)]
    last = [max(s for s in range(T) if s % NGRP == g) for g in range(NGRP)]

    with tile.TileContext(nc) as tc:
        with (
            tc.tile_pool(name="fin", bufs=1) as fp,
            tc.tile_pool(name="oh", bufs=1) as ohp,
            tc.tile_pool(name="io", bufs=3) as iop,
            tc.tile_pool(name="sq", bufs=3) as sqp,
            tc.tile_pool(name="acc", bufs=1, space="PSUM") as psa,
        ):
            final = fp.tile([P, D + K], f32)
            nc.vector.memset(final[:, :], 0.0)
            # warm the Square activation table during the first DMA
            warm = fp.tile([1, 1], f32)
            nc.scalar.activation(
                out=warm[:, :], in_=final[0:1, 0:1],
                func=mybir.ActivationFunctionType.Square,
            )

            acc = psa.tile([32 * (NGRP - 1) + C, D], f32)  # [109, 64]
            oh_all = ohp.tile([P, T * C], f8)

            step = 0
            for i, (t0, tt, owner) in enumerate(plan):
                io = iop.tile([P, TTMAX * D], f8, tag="io")
                nc.sync.dma_start(
                    out=io[:, : tt * D],
                    in_=featv[:, t0 * D : (t0 + tt) * D],
                )
                if i == 0:
                    # all one-hots in one transfer, queued right after the
                    # first feat tile (matmuls lag squares anyway)
                    nc.sync.dma_start(out=oh_all[:, :], in_=ohv[:, :])
                sq = sqp.tile([P, TTMAX * D], f8, tag="sq")
                if owner == "S":
                    nc.scalar.activation(
                        out=sq[:, : tt * D], in_=io[:, : tt * D],
                        func=mybir.ActivationFunctionType.Square,
                        accum_out=final[:, D + i : D + i + 1],
                    )
                else:
                    nc.vector.affine_mul_reduce(
                        out=sq[:, : tt * D],
                        accum_out=final[:, D + i : D + i + 1],
                        in0=io[:, : tt * D],
                        in1=io[:, : tt * D],
                        scale=1.0,
                        bias=0.0,
                    )
                for tl in range(tt):
                    g = step % NGRP
                    nc.tensor.matmul(
                        acc[32 * g : 32 * g + C, :],
                        lhsT=oh_all[:, step * C : (step + 1) * C],
                        rhs=io[:, tl * D : (tl + 1) * D],
                        start=(step == first[g]),
                        stop=(step == last[g]),
                        tile_position=(0, 32 * g),
                    )
                    step += 1

            nc.vector.tensor_copy(final[0 : 32 * 3 + C, 0:D], acc[:, :])
            nc.sync.dma_start(out=out_h[:, :], in_=final[:, :])
    nc.finalize()
    return nc


# set by test.py to capture profile info
TRACE = False
LAST = {}


def _ensure_ntff_hook():
    """The agent image's antenv lacks axon_hooks; synthesize it so
    run_bass_kernel_spmd(trace=True) can profile. Best-effort."""
    import sys
    import types

    try:
        from antenv.axon_hooks import get_axon_ntff_profile_hook  # noqa: F401
        return
    except ImportError:
        pass
    try:
        from trn_agent_boot.trn_boot import _ntff_profile_via_ctypes

        hook = _ntff_profile_via_ctypes("/opt/axon/libaxon_pjrt.so")
        mod = types.ModuleType("antenv.axon_hooks")
        mod._hook = hook
        mod.get_axon_ntff_profile_hook = lambda: mod._hook
        mod.set_axon_ntff_profile_hook = lambda h: setattr(mod, "_hook", h)
        sys.modules["antenv.axon_hooks"] = mod
        import antenv

        antenv.axon_hooks = mod
    except Exception as e:  # degrade: no profile, run still works
        print(f"ntff hook injection failed: {e}")


def kernel(pred=None, target=None, feat=None, batch=None, centers=None):
    target = np.asarray(target)
    feat = np.asarray(feat, dtype=np.float32)
    batch = np.asarray(batch)
    centers = np.asarray(centers, dtype=np.float64)

    # shard at batch boundaries: core b <- batch b (batch is sorted)
    bounds = np.searchsorted(batch, np.arange(B + 1))
    sizes = np.diff(bounds)
    T = int(max((int(sizes.max()) + P - 1) // P, 8))
    Npad = P * T

    valid_all = target != IGNORE
    feat8 = np.clip(feat, -240.0, 240.0).astype(F8NP)

    in_maps = []
    for b in range(B):
        lo, hi = int(bounds[b]), int(bounds[b + 1])
        n = hi - lo
        fb8 = np.zeros((Npad, D), dtype=F8NP)
        ohb = np.zeros((Npad, C), dtype=F8NP)
        v = valid_all[lo:hi]
        fb = feat8[lo:hi].copy()
        fb[~v] = 0
        fb8[:n] = fb
        tb = target[lo:hi]
        rows = np.nonzero(v)[0]
        ohb[rows, tb[rows]] = 1.0
        in_maps.append({"feat8": fb8, "oh8": ohb})

    nc = build_nc(T)
    if TRACE:
        _ensure_ntff_hook()
    res = run_bass_kernel_spmd(nc, in_maps, list(range(N_CORES)), trace=TRACE)
    LAST["results"] = res

    # host-side (exact, from int inputs only): per-batch/class counts
    seg = (batch.astype(np.int64) * C + np.where(valid_all, target, 0))[
        valid_all
    ]
    ccnt = np.bincount(seg, minlength=B * C).reshape(B, C).astype(np.float64)
    cnt_b = np.bincount(batch[valid_all], minlength=B).astype(np.float64)

    cen_sq = (centers**2).sum(axis=1)  # (13,)
    total_intra = 0.0
    total_inter = 0.0
    n_present = 0
    for b in range(B):
        o = np.asarray(res.results[b]["out"]).astype(np.float64)  # [128,64+K]
        fsum = np.zeros((C, D))
        for g in range(NGRP):
            fsum += o[32 * g : 32 * g + C, 0:D]
        S = o[:, D:].sum()
        if cnt_b[b] <= 0:
            continue
        n_present += 1
        tdot = (centers * fsum).sum()
        utot = (ccnt[b] * cen_sq).sum()
        total_intra += (S - 2.0 * tdot + utot) / max(cnt_b[b], 1.0)

        cm = fsum / np.maximum(ccnt[b], 1.0)[:, None]
        pres = ccnt[b] > 0
        dd2 = ((cm[:, None, :] - cm[None, :, :]) ** 2).sum(-1)
        pm = pres[:, None] & pres[None, :] & ~np.eye(C, dtype=bool)
        dist = np.sqrt(np.where(pm, dd2, 1.0))
        terms = np.where(pm, np.maximum(MARGIN - dist, 0.0), 0.0)
        npairs = pm.sum()
        total_inter += terms.sum() / max(npairs, 1)

    den = max(n_present, 1)
    loss = LOSS_W * (
        INTRA_W * total_intra / den + INTER_W * total_inter / den
    )
    return np.float32(loss)


# revision 9
# speedup vs baseline: 1.8477x; 1.0633x over previous
"""DynamicCenterLoss on Trainium2 (Bass/Tile), 8-core SPMD.

Strategy: `batch` is sorted, so shard at batch boundaries -> core b owns
batch b. Host uploads two fp8 tensors per core: feat [Npad, 64] and
onehot(target) [Npad, 13] (4x less HBM than f32 feat; no on-device cast
or broadcast-is_equal).  Device computes only the O(N) reductions:

  - fsum[13,64] per-class feature sums: per-128-point chunk matmul
    acc += onehot^T @ feat, spread over 4 PE column groups
    (tile_position) so 4 chunk-matmuls stream concurrently.
  - S = sum ||f||^2: each DMA tile is squared+reduced by ONE engine --
    ScalarE (activation Square + accum_out) or the DVE (custom-DVE
    affine_mul_reduce in0*in1 + accum; the native tensor_tensor_reduce
    ISA op crashes this silicon) -- with tile ownership balanced by the
    engines' 1.2 vs 0.96 GHz rates.  Flat step-1 access patterns.

Counts/presence (ccnt, cnt_b) are exact host-side bincounts of
target/batch; the tiny (13,64) pairwise-center tail runs on the host in
f64 as part of the gather.  fp8e4 quantization of feat gives rel err
~3.5e-4 on the loss (tolerance 2e-2).
"""

import numpy as np
import ml_dtypes

import concourse.bass as bass
import concourse.bacc as bacc
import concourse.tile as tile
from concourse import mybir
from concourse.bass_utils import run_bass_kernel_spmd

P = 128
D = 64
C = 13
B = 8
N_CORES = 8
MARGIN = 0.5
INTRA_W = 1.0
INTER_W = 1.0
LOSS_W = 0.01
IGNORE = -1
NGRP = 4  # PE column groups

f32 = mybir.dt.float32
f8 = mybir.dt.float8e4

F8NP = getattr(ml_dtypes, "float8_e4m3", ml_dtypes.float8_e4m3fn)


def _plan(T: int) -> list[tuple[int, int, str]]:
    """(t0, tt, owner) tiles. Small first tile (compute starts early),
    small last tile (less square work exposed after the DMA stream);
    owners greedy-balanced for ScalarE 1.2 GHz vs DVE 0.96 GHz."""
    szs = []
    rem = T
    for s in (32, 96):
        t = min(s, rem)
        if t:
            szs.append(t)
            rem -= t
    while rem > 160:
        szs.append(128)
        rem -= 128
    if rem > 32:
        szs.append(rem - 24)
        rem = 24
    if rem:
        szs.append(rem)

    out = []
    t0 = 0
    ts = tv = 0.0  # projected engine-busy ns
    for sz in szs:
        cs = ts + sz * D / 1.2 + 640  # init + read-accumulator
        cv = tv + sz * D / 0.96 + 250
        if cs <= cv:
            out.append((t0, sz, "S"))
            ts = cs
        else:
            out.append((t0, sz, "V"))
            tv = cv
        t0 += sz
    return out


def build_nc(T: int) -> bass.Bass:
    plan = _plan(T)
    ntiles = len(plan)
    K = ntiles  # one accumulator column per tile
    TTMAX = max(tt for _, tt, _ in plan)
    Npad = P * T

    nc = bacc.Bacc("TRN2", target_bir_lowering=False)
    feat_h = nc.dram_tensor("feat8", [Npad, D], f8, kind="ExternalInput")
    oh_h = nc.dram_tensor("oh8", [Npad, C], f8, kind="ExternalInput")
    out_h = nc.dram_tensor("out", [P, D + K], f32, kind="ExternalOutput")

    # point n == (p, t), n = p*T + t -> per-partition contiguous, flat
    featv = feat_h[:, :].rearrange("(p t) d -> p (t d)", p=P)  # [128, T*64]
    ohv = oh_h[:, :].rearrange("(p t) d -> p (t d)", p=P)  # [128, T*13]

    first = [min(s for s in range(T) if s % NGRP == g) for g in range(NGRP)]
    last = [max(s for s in range(T) if s % NGRP == g) for g in range(NGRP)]

    with tile.TileContext(nc) as tc:
        with (
            tc.tile_pool(name="fin", bufs=1) as fp,
            tc.tile_pool(name="oh", bufs=4) as ohp,
            tc.tile_pool(name="io", bufs=4) as iop,
            tc.tile_pool(name="sq", bufs=4) as sqp,
            tc.tile_pool(name="acc", bufs=1, space="PSUM") as psa,
        ):
            final = fp.tile([P, D + K], f32)
            nc.vector.memset(final[:, :], 0.0)
            # warm the Square activation table during the first DMA
            warm = fp.tile([1, 1], f32)
            nc.scalar.activation(
                out=warm[:, :], in_=final[0:1, 0:1],
                func=mybir.ActivationFunctionType.Square,
            )

            acc = psa.tile([32 * (NGRP - 1) + C, D], f32)  # [109, 64]

            step = 0
            for i, (t0, tt, owner) in enumerate(plan):
                io = iop.tile([P, TTMAX * D], f8, tag="io")
                nc.sync.dma_start(
                    out=io[:, : tt * D],
                    in_=featv[:, t0 * D : (t0 + tt) * D],
                )
                oh = ohp.tile([P, TTMAX * C], f8, tag="oh")
                nc.sync.dma_start(
                    out=oh[:, : tt * C],
                    in_=ohv[:, t0 * C : (t0 + tt) * C],
                )
                sq = sqp.tile([P, TTMAX * D], f8, tag="sq")
                if owner == "S":
                    nc.scalar.activation(
                        out=sq[:, : tt * D], in_=io[:, : tt * D],
                        func=mybir.ActivationFunctionType.Square,
                        accum_out=final[:, D + i : D + i + 1],
                    )
                else:
                    nc.vector.affine_mul_reduce(
                        out=sq[:, : tt * D],
                        accum_out=final[:, D + i : D + i + 1],
                        in0=io[:, : tt * D],
                        in1=io[:, : tt * D],
                        scale=1.0,
                        bias=0.0,
                    )
                for tl in range(tt):
                    g = step % NGRP
                    nc.tensor.matmul(
                        acc[32 * g : 32 * g + C, :],
                        lhsT=oh[:, tl * C : (tl + 1) * C],
                        rhs=io[:, tl * D : (tl + 1) * D],
                        start=(step == first[g]),
                        stop=(step == last[g]),
                        tile_position=(0, 32 * g),
                    )
                    step += 1

            nc.vector.tensor_copy(final[0 : 32 * 3 + C, 0:D], acc[:, :])
            nc.sync.dma_start(out=out_h[:, :], in_=final[:, :])
    nc.finalize()
    return nc


# set by test.py to capture profile info
TRACE = False
LAST = {}


def _ensure_ntff_hook():
    """The agent image's antenv lacks axon_hooks; synthesize it so
    run_bass_kernel_spmd(trace=True) can profile. Best-effort."""
    import sys
    import types

    try:
        from antenv.axon_hooks import get_axon_ntff_profile_hook  # noqa: F401
        return
    except ImportError:
        pass
    try:
        from trn_agent_boot.trn_boot import _ntff_profile_via_ctypes

        hook = _ntff_profile_via_ctypes("/opt/axon/libaxon_pjrt.so")
        mod = types.ModuleType("antenv.axon_hooks")
        mod._hook = hook
        mod.get_axon_ntff_profile_hook = lambda: mod._hook
        mod.set_axon_ntff_profile_hook = lambda h: setattr(mod, "_hook", h)
        sys.modules["antenv.axon_hooks"] = mod
        import antenv

        antenv.axon_hooks = mod
    except Exception as e:  # degrade: no profile, run still works
        print(f"ntff hook injection failed: {e}")


def kernel(pred=None, target=None, feat=None, batch=None, centers=None):
    target = np.asarray(target)
    feat = np.asarray(feat, dtype=np.float32)
    batch = np.asarray(batch)
    centers = np.asarray(centers, dtype=np.float64)

    # shard at batch boundaries: core b <- batch b (batch is sorted)
    bounds = np.searchsorted(batch, np.arange(B + 1))
    sizes = np.diff(bounds)
    T = int(max((int(sizes.max()) + P - 1) // P, 8))
    Npad = P * T

    valid_all = target != IGNORE
    feat8 = np.clip(feat, -240.0, 240.0).astype(F8NP)

    in_maps = []
    for b in range(B):
        lo, hi = int(bounds[b]), int(bounds[b + 1])
        n = hi - lo
        fb8 = np.zeros((Npad, D), dtype=F8NP)
        ohb = np.zeros((Npad, C), dtype=F8NP)
        v = valid_all[lo:hi]
        fb = feat8[lo:hi].copy()
        fb[~v] = 0
        fb8[:n] = fb
        tb = target[lo:hi]
        rows = np.nonzero(v)[0]
        ohb[rows, tb[rows]] = 1.0
        in_maps.append({"feat8": fb8, "oh8": ohb})

    nc = build_nc(T)
    if TRACE:
        _ensure_ntff_hook()
    res = run_bass_kernel_spmd(nc, in_maps, list(range(N_CORES)), trace=TRACE)
    LAST["results"] = res

    # host-side (exact, from int inputs only): per-batch/class counts
    seg = (batch.astype(np.int64) * C + np.where(valid_all, target, 0))[
        valid_all
    ]
    ccnt = np.bincount(seg, minlength=B * C).reshape(B, C).astype(np.float64)
    cnt_b = np.bincount(batch[valid_all], minlength=B).astype(np.float64)

    cen_sq = (centers**2).sum(axis=1)  # (13,)
    total_intra = 0.0
    total_inter = 0.0
    n_present = 0
    for b in range(B):
        o = np.asarray(res.results[b]["out"]).astype(np.float64)  # [128,64+K]
        fsum = np.zeros((C, D))
        for g in range(NGRP):
            fsum += o[32 * g : 32 * g + C, 0:D]
        S = o[:, D:].sum()
        if cnt_b[b] <= 0:
            continue
        n_present += 1
        tdot = (centers * fsum).sum()
        utot = (ccnt[b] * cen_sq).sum()
        total_intra += (S - 2.0 * tdot + utot) / max(cnt_b[b], 1.0)

        cm = fsum / np.maximum(ccnt[b], 1.0)[:, None]
        pres = ccnt[b] > 0
        dd2 = ((cm[:, None, :] - cm[None, :, :]) ** 2).sum(-1)
        pm = pres[:, None] & pres[None, :] & ~np.eye(C, dtype=bool)
        dist = np.sqrt(np.where(pm, dd2, 1.0))
        terms = np.where(pm, np.maximum(MARGIN - dist, 0.0), 0.0)
        npairs = pm.sum()
        total_inter += terms.sum() / max(npairs, 1)

    den = max(n_present, 1)
    loss = LOSS_W * (
        INTRA_W * total_intra / den + INTER_W * total_inter / den
    )
    return np.float32(loss)


# revision 12
# speedup vs baseline: 1.8572x; 1.0051x over previous
"""DynamicCenterLoss on Trainium2 (Bass/Tile), 8-core SPMD.

Strategy: `batch` is sorted, so shard at batch boundaries -> core b owns
batch b. Host uploads two fp8 tensors per core: feat [Npad, 64] and
onehot(target) [Npad, 13] (4x less HBM than f32 feat; no on-device cast
or broadcast-is_equal).  Device computes only the O(N) reductions:

  - fsum[13,64] per-class feature sums: per-128-point chunk matmul
    acc += onehot^T @ feat, spread over 4 PE column groups
    (tile_position) so 4 chunk-matmuls stream concurrently.
  - S = sum ||f||^2: each DMA tile is squared+reduced by ONE engine --
    ScalarE (activation Square + accum_out) or the DVE (custom-DVE
    affine_mul_reduce in0*in1 + accum; the native tensor_tensor_reduce
    ISA op crashes this silicon) -- with tile ownership balanced by the
    engines' 1.2 vs 0.96 GHz rates.  Flat step-1 access patterns.

Counts/presence (ccnt, cnt_b) are exact host-side bincounts of
target/batch; the tiny (13,64) pairwise-center tail runs on the host in
f64 as part of the gather.  fp8e4 quantization of feat gives rel err
~3.5e-4 on the loss (tolerance 2e-2).
"""

import numpy as np
import ml_dtypes

import concourse.bass as bass
import concourse.bacc as bacc
import concourse.tile as tile
from concourse import mybir
from concourse.bass_utils import run_bass_kernel_spmd

P = 128
D = 64
C = 13
B = 8
N_CORES = 8
MARGIN = 0.5
INTRA_W = 1.0
INTER_W = 1.0
LOSS_W = 0.01
IGNORE = -1
NGRP = 4  # PE column groups

f32 = mybir.dt.float32
f8 = mybir.dt.float8e4

F8NP = getattr(ml_dtypes, "float8_e4m3", ml_dtypes.float8_e4m3fn)


def _plan(T: int) -> list[tuple[int, int, str]]:
    """(t0, tt, owner) tiles. Small first tile (compute starts early),
    small last tile (less square work exposed after the DMA stream);
    owners greedy-balanced for ScalarE 1.2 GHz vs DVE 0.96 GHz."""
    szs = []
    rem = T
    for s in (32, 96):
        t = min(s, rem)
        if t:
            szs.append(t)
            rem -= t
    while rem > 160:
        szs.append(128)
        rem -= 128
    if rem > 32:
        szs.append(rem - 24)
        rem = 24
    if rem:
        szs.append(rem)

    out = []
    t0 = 0
    ts = tv = 0.0  # projected engine-busy ns
    for sz in szs:
        cs = ts + sz * D / 1.2 + 640  # init + read-accumulator
        cv = tv + sz * D * 1.115 / 0.96 + 250  # AMR runs ~11% below rate
        if cs <= cv:
            out.append((t0, sz, "S"))
            ts = cs
        else:
            out.append((t0, sz, "V"))
            tv = cv
        t0 += sz
    return out


def build_nc(T: int) -> bass.Bass:
    plan = _plan(T)
    ntiles = len(plan)
    K = ntiles  # one accumulator column per tile
    TTMAX = max(tt for _, tt, _ in plan)
    Npad = P * T

    nc = bacc.Bacc("TRN2", target_bir_lowering=False)
    feat_h = nc.dram_tensor("feat8", [Npad, D], f8, kind="ExternalInput")
    oh_h = nc.dram_tensor("oh8", [Npad, C], f8, kind="ExternalInput")
    out_h = nc.dram_tensor("out", [P, D + K], f32, kind="ExternalOutput")

    # point n == (p, t), n = p*T + t -> per-partition contiguous, flat
    featv = feat_h[:, :].rearrange("(p t) d -> p (t d)", p=P)  # [128, T*64]
    ohv = oh_h[:, :].rearrange("(p t) d -> p (t d)", p=P)  # [128, T*13]

    first = [min(s for s in range(T) if s % NGRP == g) for g in range(NGRP)]
    last = [max(s for s in range(T) if s % NGRP == g) for g in range(NGRP)]

    with tile.TileContext(nc) as tc:
        with (
            tc.tile_pool(name="fin", bufs=1) as fp,
            tc.tile_pool(name="oh", bufs=1) as ohp,
            tc.tile_pool(name="io", bufs=4) as iop,
            tc.tile_pool(name="sq", bufs=4) as sqp,
            tc.tile_pool(name="acc", bufs=1, space="PSUM") as psa,
        ):
            final = fp.tile([P, D + K], f32)
            nc.vector.memset(final[:, :], 0.0)

            # all one-hots in one transfer on the scalar-engine HWDGE queue
            # so it drains in parallel with the feat stream on the sync queue
            oh_all = ohp.tile([P, T * C], f8)
            nc.scalar.dma_start(out=oh_all[:, :], in_=ohv[:, :])
            # warm the Square activation table during the first DMA
            warm = fp.tile([1, 1], f32)
            nc.scalar.activation(
                out=warm[:, :], in_=final[0:1, 0:1],
                func=mybir.ActivationFunctionType.Square,
            )

            acc = psa.tile([32 * (NGRP - 1) + C, D], f32)  # [109, 64]

            step = 0
            for i, (t0, tt, owner) in enumerate(plan):
                io = iop.tile([P, TTMAX * D], f8, tag="io")
                nc.sync.dma_start(
                    out=io[:, : tt * D],
                    in_=featv[:, t0 * D : (t0 + tt) * D],
                )
                sq = sqp.tile([P, TTMAX * D], f8, tag="sq")
                if owner == "S":
                    nc.scalar.activation(
                        out=sq[:, : tt * D], in_=io[:, : tt * D],
                        func=mybir.ActivationFunctionType.Square,
                        accum_out=final[:, D + i : D + i + 1],
                    )
                else:
                    nc.vector.affine_mul_reduce(
                        out=sq[:, : tt * D],
                        accum_out=final[:, D + i : D + i + 1],
                        in0=io[:, : tt * D],
                        in1=io[:, : tt * D],
                        scale=1.0,
                        bias=0.0,
                    )
                for tl in range(tt):
                    g = step % NGRP
                    nc.tensor.matmul(
                        acc[32 * g : 32 * g + C, :],
                        lhsT=oh_all[:, step * C : (step + 1) * C],
                        rhs=io[:, tl * D : (tl + 1) * D],
                        start=(step == first[g]),
                        stop=(step == last[g]),
                        tile_position=(0, 32 * g),
                    )
                    step += 1

            nc.vector.tensor_copy(final[0 : 32 * 3 + C, 0:D], acc[:, :])
            nc.sync.dma_start(out=out_h[:, :], in_=final[:, :])
    nc.finalize()
    return nc


# set by test.py to capture profile info
TRACE = False
LAST = {}


def _ensure_ntff_hook():
    """The agent image's antenv lacks axon_hooks; synthesize it so
    run_bass_kernel_spmd(trace=True) can profile. Best-effort."""
    import sys
    import types

    try:
        from antenv.axon_hooks import get_axon_ntff_profile_hook  # noqa: F401
        return
    except ImportError:
        pass
    try:
        from trn_agent_boot.trn_boot import _ntff_profile_via_ctypes

        hook = _ntff_profile_via_ctypes("/opt/axon/libaxon_pjrt.so")
        mod = types.ModuleType("antenv.axon_hooks")
        mod._hook = hook
        mod.get_axon_ntff_profile_hook = lambda: mod._hook
        mod.set_axon_ntff_profile_hook = lambda h: setattr(mod, "_hook", h)
        sys.modules["antenv.axon_hooks"] = mod
        import antenv

        antenv.axon_hooks = mod
    except Exception as e:  # degrade: no profile, run still works
        print(f"ntff hook injection failed: {e}")


def kernel(pred=None, target=None, feat=None, batch=None, centers=None):
    target = np.asarray(target)
    feat = np.asarray(feat, dtype=np.float32)
    batch = np.asarray(batch)
    centers = np.asarray(centers, dtype=np.float64)

    # shard at batch boundaries: core b <- batch b (batch is sorted)
    bounds = np.searchsorted(batch, np.arange(B + 1))
    sizes = np.diff(bounds)
    T = int(max((int(sizes.max()) + P - 1) // P, 8))
    Npad = P * T

    valid_all = target != IGNORE
    feat8 = np.clip(feat, -240.0, 240.0).astype(F8NP)

    in_maps = []
    for b in range(B):
        lo, hi = int(bounds[b]), int(bounds[b + 1])
        n = hi - lo
        fb8 = np.zeros((Npad, D), dtype=F8NP)
        ohb = np.zeros((Npad, C), dtype=F8NP)
        v = valid_all[lo:hi]
        fb = feat8[lo:hi].copy()
        fb[~v] = 0
        fb8[:n] = fb
        tb = target[lo:hi]
        rows = np.nonzero(v)[0]
        ohb[rows, tb[rows]] = 1.0
        in_maps.append({"feat8": fb8, "oh8": ohb})

    nc = build_nc(T)
    if TRACE:
        _ensure_ntff_hook()
    res = run_bass_kernel_spmd(nc, in_maps, list(range(N_CORES)), trace=TRACE)
    LAST["results"] = res

    # host-side (exact, from int inputs only): per-batch/class counts
    seg = (batch.astype(np.int64) * C + np.where(valid_all, target, 0))[
        valid_all
    ]
    ccnt = np.bincount(seg, minlength=B * C).reshape(B, C).astype(np.float64)
    cnt_b = np.bincount(batch[valid_all], minlength=B).astype(np.float64)

    cen_sq = (centers**2).sum(axis=1)  # (13,)
    total_intra = 0.0
    total_inter = 0.0
    n_present = 0
    for b in range(B):
        o = np.asarray(res.results[b]["out"]).astype(np.float64)  # [128,64+K]
        fsum = np.zeros((C, D))
        for g in range(NGRP):
            fsum += o[32 * g : 32 * g + C, 0:D]
        S = o[:, D:].sum()
        if cnt_b[b] <= 0:
            continue
        n_present += 1
        tdot = (centers * fsum).sum()
        utot = (ccnt[b] * cen_sq).sum()
        total_intra += (S - 2.0 * tdot + utot) / max(cnt_b[b], 1.0)

        cm = fsum / np.maximum(ccnt[b], 1.0)[:, None]
        pres = ccnt[b] > 0
        dd2 = ((cm[:, None, :] - cm[None, :, :]) ** 2).sum(-1)
        pm = pres[:, None] & pres[None, :] & ~np.eye(C, dtype=bool)
        dist = np.sqrt(np.where(pm, dd2, 1.0))
        terms = np.where(pm, np.maximum(MARGIN - dist, 0.0), 0.0)
        npairs = pm.sum()
        total_inter += terms.sum() / max(npairs, 1)

    den = max(n_present, 1)
    loss = LOSS_W * (
        INTRA_W * total_intra / den + INTER_W * total_inter / den
    )
    return np.float32(loss)
